# revision 38
# baseline (speedup 1.0000x reference)
"""Multi-head attention (B=2, S=2048, D=1024, H=16) on 8 Trainium2 NeuronCores.

Sharding: batch x head-group. Core c handles batch c//4 and heads 4*(c%4)..4*(c%4)+3
(column-parallel Wq/Wk/Wv, row-parallel Wo; partial outputs summed on host).

v3 schedule. The stream is jointly paced by the Scalar engine's exp stream
(~2.1us/slot) and the PE (~2.4us/slot incl. projections); everything else is
arranged so neither ever waits on a third party:
  - head: Wk/Wq ride the scalar queue (its only DMAs), X^T K0/Q0 chunks are
    split in halves so projections start on the first half; warmup is 10
    matmuls (just enough to cover the first DMA and ramp the PE clock)
  - psp PSUM ring (2x2 banks) carries scores only; ACT ping-pongs it with the
    PE. All filler work (V/Q2/Q3 projections, out-projections) threads
    through the pvp ring's idle windows or rides psp between slots, sized so
    each unit's drain (cast) completes before the ring comes around.
  - engine load-balance: DVE keeps the mask multiply, kt/qt casts, recip and
    the attnT normalize; GpSimd takes V-proj casts, the softmax-sums copy,
    partition broadcast, half of each out-proj cast and most DMA issues;
    Scalar runs activations only.
  - out-projections are spread mid-stream (one per odd slot in pvp idle
    windows); the last four are c-split so their first half runs before the
    final flush.
bq/bk are structurally zero in the reference; bv/bo are folded in on host
(sum_k softmax = 1 makes bv@Wo a constant row).
"""

import numpy as np
import ml_dtypes

B, S, D, H, HD = 2, 2048, 1024, 16, 64
NCORES = 8
HPC = 4          # heads per core
DH4 = HPC * HD   # 256 projection cols per core
KCP = D // 128   # 8 contraction chunks for projections
SC = S // 512    # 4 sq chunks
KCS = S // 128   # 16 sk chunks

_CACHE = {}


def _build_nc():
    from contextlib import ExitStack

    import concourse.bacc as bacc
    import concourse.tile as tile
    from concourse import mybir

    dt = mybir.dt
    AF = mybir.ActivationFunctionType

    nc = bacc.Bacc("TRN2", target_bir_lowering=False, debug=False)

    xT = [
        nc.dram_tensor(n, [128, SC, KCP, 512], dt.bfloat16, kind="ExternalInput")
        for n in ("xqT", "xkT", "xvT")
    ]
    maskT_d = nc.dram_tensor(
        "maskT", [128, SC, KCS, 512], dt.bfloat16, kind="ExternalInput"
    )
    wqkv_d = nc.dram_tensor(
        "wqkv", [128, 3, KCP, DH4], dt.bfloat16, kind="ExternalInput"
    )
    wo_d = nc.dram_tensor("wo", [128, 2, D], dt.bfloat16, kind="ExternalInput")
    out_d = nc.dram_tensor("out", [S, D], dt.bfloat16, kind="ExternalOutput")

    with tile.TileContext(nc) as tc, ExitStack() as ctx:
        consts = ctx.enter_context(tc.tile_pool(name="consts", bufs=1))
        wpool = ctx.enter_context(tc.tile_pool(name="wpool", bufs=1))
        persist = ctx.enter_context(tc.tile_pool(name="persist", bufs=1))
        xtpool = ctx.enter_context(tc.tile_pool(name="xtpool", bufs=3))
        xvpool = ctx.enter_context(tc.tile_pool(name="xvpool", bufs=3))
        maskpool = ctx.enter_context(tc.tile_pool(name="maskpool", bufs=3))
        ptpool = ctx.enter_context(tc.tile_pool(name="ptpool", bufs=13))
        smalls = ctx.enter_context(tc.tile_pool(name="smalls", bufs=2))
        outpool = ctx.enter_context(tc.tile_pool(name="outpool", bufs=2))
        psp = ctx.enter_context(tc.tile_pool(name="psp", bufs=2, space="PSUM"))
        pvp = ctx.enter_context(tc.tile_pool(name="pvp", bufs=2, space="PSUM"))

        # ---- upfront DMA issues ----
        # the scalar queue's few issues all happen before its first ACTIVATE,
        # so it is free for the activation stream afterwards
        w_sb = wpool.tile([128, 3, KCP, DH4], dt.bfloat16, tag="w")
        nc.scalar.dma_start(out=w_sb[:, 1, :, :], in_=wqkv_d[:, 1, :, :])  # Wk
        wo_sb = consts.tile([128, 2, D], dt.bfloat16, tag="wo")

        xk_t = [None] * SC
        xq_t = [None] * SC
        xv_t = [None] * SC

        def dma_x(eng, lst, t, sc, halves=False):
            lst[sc] = xtv = (xtpool if t != 2 else xvpool).tile(
                [128, KCP, 512], dt.bfloat16,
                tag=("xt" if t != 2 else "xv"),
                name=f"x{'qkv'[t]}{sc}",
            )
            if halves:
                h = KCP // 2
                eng.dma_start(out=xtv[:, 0:h, :], in_=xT[t][:, sc, 0:h, :])
                eng.dma_start(out=xtv[:, h:KCP, :], in_=xT[t][:, sc, h:KCP, :])
            else:
                eng.dma_start(out=xtv[:, :, :], in_=xT[t][:, sc, :, :])

        # mask halves: (sc, h) covers kc2 in [4h, 4h+4) of both p-groups of sc
        mhalf = {}

        def dma_mask(sc, h, eng):
            mhalf[(sc, h)] = mt = maskpool.tile(
                [128, KCS // 2, 512], dt.bfloat16, tag="mk", name=f"mk{sc}_{h}"
            )
            eng.dma_start(
                out=mt[:, :, :], in_=maskT_d[:, sc, 8 * h : 8 * h + 8, :]
            )

        # Queues are FIFO and share HBM bandwidth roughly equally while all
        # are draining, so each queue carries its own stream in consumption
        # order: sync the K/Q tiles (first ones split in halves so the
        # projections start on the first 4 kc chunks), scalar only the two
        # head weights, gpsimd mask + V + late weights.
        # sync carries the PE-critical K/Q stream plus late V chunks in
        # deadline order; the short gpsimd queue (mask + first V) stops
        # competing for HBM after ~30us, freeing bandwidth for xk2/xk3
        nc.scalar.dma_start(out=w_sb[:, 0, :, :], in_=wqkv_d[:, 0, :, :])  # Wq
        dma_x(nc.sync, xk_t, 1, 0, halves=True)
        dma_x(nc.sync, xq_t, 0, 0, halves=True)
        dma_mask(0, 0, nc.gpsimd)
        nc.gpsimd.dma_start(out=w_sb[:, 2, :, :], in_=wqkv_d[:, 2, :, :])  # Wv
        dma_x(nc.gpsimd, xv_t, 2, 0)  # xv0 first: it must head the xv ring
        dma_x(nc.sync, xk_t, 1, 1)
        dma_x(nc.sync, xk_t, 1, 2)
        dma_x(nc.sync, xk_t, 1, 3)
        dma_x(nc.sync, xq_t, 0, 1)
        dma_x(nc.sync, xv_t, 2, 1)
        dma_x(nc.sync, xv_t, 2, 2)
        dma_x(nc.sync, xq_t, 0, 2)
        dma_x(nc.sync, xv_t, 2, 3)
        dma_x(nc.sync, xq_t, 0, 3)
        dma_mask(0, 1, nc.gpsimd)
        dma_mask(1, 0, nc.gpsimd)
        nc.sync.dma_start(out=wo_sb[:, :, :], in_=wo_d[:, :, :])

        # ---- persistent SBUF ----
        qt_c = [
            persist.tile([128, 2, 512], dt.bfloat16, tag=f"qt{i}", name=f"qt{i}")
            for i in range(SC)
        ]
        kt_c = [
            persist.tile([128, 2, 512], dt.bfloat16, tag=f"kt{i}", name=f"kt{i}")
            for i in range(SC)
        ]
        vaug_c = [
            persist.tile(
                [128, 4, HPC, HD + 1], dt.bfloat16, tag=f"va{i}", name=f"va{i}"
            )
            for i in range(SC)
        ]
        attnT = persist.tile([128, 2, S], dt.bfloat16, tag="attnT")
        for i in range(SC):
            nc.vector.memset(vaug_c[i][:, :, :, HD : HD + 1], 1.0)

        # PE warm-up: enough zero matmuls to ramp the clock and cover the
        # first input DMA; the real projections keep the ramp going
        warm = consts.tile([128, 640], dt.bfloat16, tag="warm")
        nc.vector.memset(warm[:, :], 0.0)
        ps_w = psp.tile([128, 1024], dt.float32, tag="ps", name="warmps")
        for i in range(6):
            nc.tensor.matmul(
                ps_w[:, 0:512],
                lhsT=warm[:, 0:128],
                rhs=warm[:, 128:640],
                start=(i == 0),
                stop=(i == 5),
            )

        def proj_qk(t, sc, cast_on_scalar=False):
            # Q^T/K^T chunk: lhsT = W chunk (stationary), rhs = X^T chunk
            src = (xq_t if t == 0 else xk_t)[sc]
            dst = (qt_c, kt_c)[t][sc]
            ps = pvp.tile([128, 1024], dt.float32, tag="pv", name=f"pj{t}{sc}")
            for kc in range(KCP):
                for m in range(2):
                    nc.tensor.matmul(
                        ps[:, m * 512 : (m + 1) * 512],
                        lhsT=w_sb[:, t, kc, m * 128 : (m + 1) * 128],
                        rhs=src[:, kc, :],
                        start=(kc == 0),
                        stop=(kc == KCP - 1),
                    )
            if cast_on_scalar:
                # pre-first-ACT head bubble: the Scalar engine is idle
                nc.scalar.activation(
                    out=dst[:, :, :],
                    in_=ps[:, :].rearrange("s (m q) -> s m q", m=2),
                    func=AF.Copy,
                )
            else:
                nc.vector.tensor_copy(
                    out=dst[:, :, :],
                    in_=ps[:, :].rearrange("s (m q) -> s m q", m=2),
                )

        # quarter-granular Q projection for the late Q2/Q3 (threads through
        # the pvp ring without monopolizing the PE for 3.5us)
        qproj_state = {}

        def proj_q_quarter(sc, quarter):
            if quarter == 0:
                qproj_state[sc] = pvp.tile(
                    [128, 1024], dt.float32, tag="pv", name=f"pjq{sc}"
                )
            ps = qproj_state[sc]
            for kc in range(2 * quarter, 2 * quarter + 2):
                for m in range(2):
                    nc.tensor.matmul(
                        ps[:, m * 512 : (m + 1) * 512],
                        lhsT=w_sb[:, 0, kc, m * 128 : (m + 1) * 128],
                        rhs=xq_t[sc][:, kc, :],
                        start=(kc == 0),
                        stop=(kc == KCP - 1),
                    )
            if quarter == 3:
                nc.vector.tensor_copy(
                    out=qt_c[sc][:, :, :],
                    in_=ps[:, :].rearrange("s (m q) -> s m q", m=2),
                )

        def proj_v_unit(sc, j, pool):
            # one V 128-row chunk: lhsT = X_v^T chunk (stationary), rhs = W_v
            po_v = pool.tile(
                [128, DH4], dt.float32,
                tag=("pv" if pool is pvp else "ps"),
                name=f"pV{sc}{j}",
            )
            for kc in range(KCP):
                nc.tensor.matmul(
                    po_v[:, :],
                    lhsT=xv_t[sc][:, kc, j * 128 : (j + 1) * 128],
                    rhs=w_sb[:, 2, kc, :],
                    start=(kc == 0),
                    stop=(kc == KCP - 1),
                )
            nc.vector.tensor_copy(
                out=vaug_c[sc][:, j, :, 0:HD],
                in_=po_v[:, :].rearrange("p (h d) -> p h d", h=4),
            )

        # slot stream: scores/exp/mask for slot s; AV trails (paced below).
        # Within each sc the kc2 range is split in halves with both parities
        # interleaved, so the first 8 slots of an sc touch only the first two
        # kt chunks — the later K chunks' DMA+projection deadlines move ~4
        # slots (~9us) later, which removes the early-phase scores stalls.
        SLOTS = [
            (sc, p, 4 * half + q)
            for sc in range(SC)
            for half in range(2)
            for p in range(2)
            for q in range(4)
        ]
        pt_ring = [None] * len(SLOTS)
        pv_cur = [None, None]  # pv psum per group parity

        def scores_block(s):
            sc, p, kc2 = SLOTS[s]
            pt = ptpool.tile([128, 2, 2, 512], dt.bfloat16, tag="pt")
            pt_ring[s] = pt
            for j in range(2):
                kc = 2 * kc2 + j
                ps = psp.tile([128, 1024], dt.float32, tag="ps")
                nc.tensor.matmul(
                    ps[:, 0:512],
                    lhsT=kt_c[kc // 4][
                        0:64, p, (kc % 4) * 128 : (kc % 4 + 1) * 128
                    ],
                    rhs=qt_c[sc][0:64, p, :],
                    start=True,
                    stop=True,
                )
                nc.tensor.matmul(
                    ps[:, 512:1024],
                    lhsT=kt_c[kc // 4][
                        64:128, p, (kc % 4) * 128 : (kc % 4 + 1) * 128
                    ],
                    rhs=qt_c[sc][64:128, p, :],
                    start=True,
                    stop=True,
                    tile_position=(64, 0),
                )
                nc.scalar.activation(
                    out=pt[:, :, j, :],
                    in_=ps[:, :].rearrange("s (h q) -> s h q", h=2),
                    func=AF.Exp,
                    scale=0.125,
                )

        def mask_mul(s):
            # deferred ~2 slots behind the exp stream: AV only needs pt ~4+
            # slots later, and the lower DVE priority lets flush copies (the
            # pv ring handover) jump the queue
            sc, p, kc2 = SLOTS[s]
            pt = pt_ring[s]
            mt = mhalf[(sc, kc2 // 4)]
            msl = mt[:, 2 * (kc2 % 4) : 2 * (kc2 % 4) + 2, :]
            nc.vector.tensor_mul(
                out=pt[:, :, :, :],
                in0=pt[:, :, :, :],
                in1=msl.unsqueeze(1).broadcast_to([128, 2, 2, 512]),
            )

        def av_block(s):
            sc, p, kc2 = SLOTS[s]
            if kc2 == 0:
                pv_cur[p] = pvp.tile(
                    [HD + 1, 1024], dt.float32, tag="pv", name=f"pv{2 * sc + p}"
                )
            pv = pv_cur[p]
            pt = pt_ring[s]
            for j in range(2):
                kc = 2 * kc2 + j
                for i in range(2):
                    nc.tensor.matmul(
                        pv[:, i * 512 : (i + 1) * 512],
                        lhsT=vaug_c[kc // 4][:, kc % 4, 2 * p + i, :],
                        rhs=pt[:, i, j, :],
                        start=(kc == 0),
                        stop=(kc == KCS - 1),
                    )
            pt_ring[s] = None

        ones_sb = consts.tile([1, 64], dt.bfloat16, tag="ones")
        nc.vector.memset(ones_sb[:, :], 1.0)

        def flush(g, pe_bcast=False):
            sc, p = g // 2, g % 2
            pv = pv_cur[g % 2]
            # Stage the pv group to SBUF immediately: the two copies are
            # pv's only readers, so the PSUM slot recycles ~2us sooner than
            # if the muls (queued behind recip+broadcast) read it, and the
            # muls keep a single-PSUM-operand access pattern.
            stage = smalls.tile(
                [HD, 1024], dt.float32, tag="stage", name=f"stage{g}",
                bufs=1,
            )
            nc.vector.tensor_copy(out=stage[:, :], in_=pv[0:HD, :])
            sums_sb = smalls.tile(
                [1, 1024], dt.float32, tag="sums", name=f"sums{g}"
            )
            # custom DVE ops drop the input base-partition: stage the sums
            # row to partition 0 in SBUF before the approx reciprocal.
            nc.vector.tensor_copy(out=sums_sb[0:1, :], in_=pv[HD : HD + 1, :])
            recip_sb = smalls.tile(
                [1, 1024], dt.float32, tag="recip", name=f"recip{g}"
            )
            nc.vector.reciprocal_approx_fast(
                out=recip_sb[0:1, :], in_=sums_sb[0:1, :]
            )
            if pe_bcast:
                # tail only (PSUM is free then): rank-1 matmul broadcast is
                # ~4x faster than the GpSimd partition broadcast
                bcs = pvp.tile(
                    [64, 1024], dt.float32, tag="pv", name=f"bcsp{g}"
                )
                rb = smalls.tile(
                    [1, 1024], dt.bfloat16, tag="recipb", name=f"recipb{g}",
                    bufs=1,
                )
                nc.vector.tensor_copy(out=rb[0:1, :], in_=recip_sb[0:1, :])
                for m in range(2):
                    nc.tensor.matmul(
                        bcs[:, m * 512 : (m + 1) * 512],
                        lhsT=ones_sb[0:1, :],
                        rhs=rb[0:1, m * 512 : (m + 1) * 512],
                        start=True,
                        stop=True,
                    )
            else:
                bcs = smalls.tile(
                    [64, 1024], dt.float32, tag="bcs", name=f"bcs{g}"
                )
                nc.gpsimd.partition_broadcast(bcs[:, :], recip_sb[0:1, :])
            for i in range(2):
                nc.vector.tensor_mul(
                    out=attnT[
                        64 * i : 64 * (i + 1), p, sc * 512 : (sc + 1) * 512
                    ],
                    in0=stage[:, i * 512 : (i + 1) * 512],
                    in1=bcs[0:HD, i * 512 : (i + 1) * 512],
                )

        po_state = {}

        def po_mm(s1, pool, cs=(0, 1)):
            # out-projection matmuls for sq chunk s1, contraction parts cs
            if 0 in cs:
                po_state[s1] = pool.tile(
                    [128, 1024], dt.float32,
                    tag=("pv" if pool is pvp else "ps"),
                    name=f"po{s1}",
                )
            po = po_state[s1]
            for c in cs:
                for m in range(2):
                    nc.tensor.matmul(
                        po[:, m * 512 : (m + 1) * 512],
                        lhsT=attnT[:, c, s1 * 128 : (s1 + 1) * 128],
                        rhs=wo_sb[:, c, m * 512 : (m + 1) * 512],
                        start=(c == 0),
                        stop=(c == 1),
                    )

        def po_out(s1, dma_eng, scalar_cast=False):
            # two half casts: finer DVE granularity so flush copies (the pv
            # ring handover) never queue behind a full 1024-wide cast. At the
            # tail the idle Scalar engine takes alternate casts whole.
            po = po_state.pop(s1)
            ot = outpool.tile([128, 1024], dt.bfloat16, tag="ot")
            if scalar_cast:
                nc.scalar.activation(out=ot[:, :], in_=po[:, :], func=AF.Copy)
            else:
                nc.vector.tensor_copy(out=ot[:, 0:512], in_=po[:, 0:512])
                nc.vector.tensor_copy(out=ot[:, 512:1024], in_=po[:, 512:1024])
            dma_eng.dma_start(
                out=out_d[s1 * 128 : (s1 + 1) * 128, :], in_=ot[:, :]
            )

        def out_proj_one(s1, pool, dma_eng=None, scalar_cast=False):
            po_mm(s1, pool)
            po_out(
                s1,
                dma_eng if dma_eng is not None else nc.gpsimd,
                scalar_cast=scalar_cast,
            )

        MASK_LATE = [(1, 1), (2, 0), (2, 1), (3, 0), (3, 1)]

        def group_done(g):
            flush(g, pe_bcast=(g == 7))
            if g < len(MASK_LATE):
                dma_mask(*MASK_LATE[g], nc.sync)

        # ---- static filler schedule (see module docstring) ----
        # Each entry is (slot -> list of closures); units are sized ~<1us of
        # PE so they thread through ring idle windows without stalling ACT.
        fillers = {}

        def add_fill(s, fn):
            fillers.setdefault(s, []).append(fn)

        # V projections: 16 units across slots 8-16; the last three ride psp
        # only (both pvp slots are group-occupied when they are due)
        VFILL = [(0, 0, 8, pvp), (0, 1, 8, psp), (0, 2, 9, pvp), (0, 3, 9, psp),
                 (1, 0, 10, pvp), (1, 1, 10, psp), (1, 2, 11, pvp), (1, 3, 11, psp),
                 (2, 0, 12, pvp), (2, 1, 12, psp), (2, 2, 13, pvp), (2, 3, 13, psp),
                 (3, 0, 14, pvp), (3, 1, 14, psp), (3, 2, 15, psp), (3, 3, 16, psp)]
        for scv, j, s_at, pool in VFILL:
            add_fill(s_at, (lambda sc_, j_, p_: lambda: proj_v_unit(sc_, j_, p_))(scv, j, pool))
        # late Q projections thread the pvp ring in quarter units
        for q in range(4):
            add_fill(16 + q, (lambda q_: lambda: proj_q_quarter(2, q_))(q))
        for q in range(4):
            add_fill(37 + q, (lambda q_: lambda: proj_q_quarter(3, q_))(q))
        # out-projections mid-stream in pvp/psp idle windows
        for s1, s_at in [(0, 29), (1, 31), (2, 33), (3, 35),
                         (4, 45), (5, 47), (6, 49),
                         (7, 53), (8, 55), (9, 57)]:
            add_fill(s_at, (lambda s1_: lambda: out_proj_one(s1_, pvp))(s1))
        for s1, s_at in [(10, 59), (11, 61)]:
            add_fill(s_at, (lambda s1_: lambda: out_proj_one(s1_, psp))(s1))

        def av_target(s):
            t = s - 10 if s < 18 else 2 * s - 28
            # wind the lag down at the end; never below 2 (mask_mul of slot
            # s-2 is the freshest one emitted before this slot's av drain)
            lag = 4 if s < 61 else max(2, 64 - s)
            return min(t, s - lag, len(SLOTS) - 1)

        # ---- program order (= scheduler priority) ----
        proj_qk(1, 0, cast_on_scalar=True)   # K0
        proj_qk(0, 0, cast_on_scalar=True)   # Q0: first scores need only kt0+qt0
        scores_block(0)
        mask_mul(0)
        scores_block(1)
        mask_mul(1)
        proj_qk(1, 1)            # K1
        scores_block(2)
        mask_mul(2)
        scores_block(3)
        mask_mul(3)
        proj_qk(1, 2)            # K2
        scores_block(4)
        mask_mul(4)
        scores_block(5)
        mask_mul(5)
        proj_qk(1, 3)            # K3
        scores_block(6)
        proj_qk(0, 1)            # Q1 (PE slack in this window)
        scores_block(7)
        av_next = 0
        for s in range(8, len(SLOTS)):
            scores_block(s)
            for fn in fillers.get(s, ()):
                fn()
            mask_mul(s - 2)
            target = av_target(s)
            while av_next <= target:
                av_block(av_next)
                sc_a, p_a, kc2_a = SLOTS[av_next]
                if kc2_a == 7:
                    group_done(2 * sc_a + p_a)
                av_next += 1
        # ---- tail ----
        # first contraction halves of the last out-proj chunks run before the
        # final flush (keeping the PE busy and at clock through it); second
        # halves + the last chunks trail it
        mask_mul(62)
        po_mm(12, psp, cs=(0,))
        mask_mul(63)
        po_mm(13, psp, cs=(0,))
        po_mm(14, pvp, cs=(0,))
        while av_next < len(SLOTS):
            av_block(av_next)
            sc_a, p_a, kc2_a = SLOTS[av_next]
            if kc2_a == 7:
                group_done(2 * sc_a + p_a)
            av_next += 1
        po_mm(12, psp, cs=(1,))
        po_out(12, nc.sync)
        po_mm(13, psp, cs=(1,))
        po_out(13, nc.scalar, scalar_cast=True)
        po_mm(14, pvp, cs=(1,))
        po_out(14, nc.sync)
        out_proj_one(15, psp, nc.scalar, scalar_cast=True)

    nc.compile()
    return nc


def _prep_inputs(query, key_, value, mask, Wq, bq, Wk, bk, Wv, bv, Wo, bo):
    bf16 = ml_dtypes.bfloat16
    f32 = np.float32

    def _xblock(x):
        # [S, D] -> X^T [D, S] -> [128p, SC, KCP, 512] (contiguous per partition)
        xt = np.asarray(x, f32).T.astype(bf16)
        return np.ascontiguousarray(
            xt.reshape(KCP, 128, SC, 512).transpose(1, 2, 0, 3)
        )

    def _mblock(mk):
        mt = np.asarray(mk).T.astype(bf16)  # maskT [sk, sq]
        return np.ascontiguousarray(
            mt.reshape(KCS, 128, SC, 512).transpose(1, 2, 0, 3)
        )

    per_batch = []
    for b in range(B):
        per_batch.append(
            {
                "xqT": _xblock(query[b]),
                "xkT": _xblock(key_[b]),
                "xvT": _xblock(value[b]),
                "maskT": _mblock(mask[b, 0]),
            }
        )
    in_maps = []
    for c in range(NCORES):
        b, hq = divmod(c, NCORES // B)
        cs = slice(DH4 * hq, DH4 * (hq + 1))
        m = dict(per_batch[b])

        def _wblock(w):
            ws = np.asarray(w, f32)[:, cs].astype(bf16)  # [D, 256]
            return ws.reshape(KCP, 128, DH4).transpose(1, 0, 2)

        m["wqkv"] = np.ascontiguousarray(
            np.stack([_wblock(Wq), _wblock(Wk), _wblock(Wv)], axis=1)
        )  # [128, 3, KCP, DH4]
        wos = np.asarray(Wo, f32)[cs, :].astype(bf16)  # [256, D]
        m["wo"] = np.ascontiguousarray(wos.reshape(2, 128, D).transpose(1, 0, 2))
        in_maps.append(m)
    return in_maps


def kernel(query, key_, value, mask, Wq, bq, Wk, bk, Wv, bv, Wo, bo):
    from concourse.bass_utils import run_bass_kernel_spmd

    if "nc" not in _CACHE:
        _CACHE["nc"] = _build_nc()
    nc = _CACHE["nc"]

    in_maps = _prep_inputs(
        query, key_, value, mask, Wq, bq, Wk, bk, Wv, bv, Wo, bo
    )
    res = run_bass_kernel_spmd(nc, in_maps, core_ids=list(range(NCORES))).results

    out = np.zeros((B, S, D), np.float32)
    for c in range(NCORES):
        out[c // (NCORES // B)] += res[c]["out"].astype(np.float32)
    out += (
        np.asarray(bv, np.float32) @ np.asarray(Wo, np.float32)
        + np.asarray(bo, np.float32)
    )[None, None, :]
    return out


# revision 40
# speedup vs baseline: 1.0876x; 1.0876x over previous
"""Multi-head attention (B=2, S=2048, D=1024, H=16) on 8 Trainium2 NeuronCores.

Sharding: batch x head-group. Core c handles batch c//4 and heads 4*(c%4)..4*(c%4)+3
(column-parallel Wq/Wk/Wv, row-parallel Wo; partial outputs summed on host).

v3 schedule. The stream is jointly paced by the Scalar engine's exp stream
(~2.1us/slot) and the PE (~2.4us/slot incl. projections); everything else is
arranged so neither ever waits on a third party:
  - head: Wk/Wq ride the scalar queue (its only DMAs), X^T K0/Q0 chunks are
    split in halves so projections start on the first half; warmup is 10
    matmuls (just enough to cover the first DMA and ramp the PE clock)
  - psp PSUM ring (2x2 banks) carries scores only; ACT ping-pongs it with the
    PE. All filler work (V/Q2/Q3 projections, out-projections) threads
    through the pvp ring's idle windows or rides psp between slots, sized so
    each unit's drain (cast) completes before the ring comes around.
  - engine load-balance: DVE keeps the mask multiply, kt/qt casts, recip and
    the attnT normalize; GpSimd takes V-proj casts, the softmax-sums copy,
    partition broadcast, half of each out-proj cast and most DMA issues;
    Scalar runs activations only.
  - out-projections are spread mid-stream (one per odd slot in pvp idle
    windows); the last four are c-split so their first half runs before the
    final flush.
bq/bk are structurally zero in the reference; bv/bo are folded in on host
(sum_k softmax = 1 makes bv@Wo a constant row).
"""

import numpy as np
import ml_dtypes

B, S, D, H, HD = 2, 2048, 1024, 16, 64
NCORES = 8
HPC = 4          # heads per core
DH4 = HPC * HD   # 256 projection cols per core
KCP = D // 128   # 8 contraction chunks for projections
SC = S // 512    # 4 sq chunks
KCS = S // 128   # 16 sk chunks

_CACHE = {}


def _build_nc():
    from contextlib import ExitStack

    import concourse.bacc as bacc
    import concourse.tile as tile
    from concourse import mybir

    dt = mybir.dt
    AF = mybir.ActivationFunctionType

    nc = bacc.Bacc("TRN2", target_bir_lowering=False, debug=False)

    xT = [
        nc.dram_tensor(n, [128, SC, KCP, 512], dt.bfloat16, kind="ExternalInput")
        for n in ("xqT", "xkT", "xvT")
    ]
    maskT_d = nc.dram_tensor(
        "maskT", [128, SC, KCS, 512], dt.bfloat16, kind="ExternalInput"
    )
    wqkv_d = nc.dram_tensor(
        "wqkv", [128, 3, KCP, DH4], dt.bfloat16, kind="ExternalInput"
    )
    wo_d = nc.dram_tensor("wo", [128, 2, D], dt.bfloat16, kind="ExternalInput")
    out_d = nc.dram_tensor("out", [S, D], dt.bfloat16, kind="ExternalOutput")

    with tile.TileContext(nc) as tc, ExitStack() as ctx:
        consts = ctx.enter_context(tc.tile_pool(name="consts", bufs=1))
        wpool = ctx.enter_context(tc.tile_pool(name="wpool", bufs=1))
        persist = ctx.enter_context(tc.tile_pool(name="persist", bufs=1))
        xtpool = ctx.enter_context(tc.tile_pool(name="xtpool", bufs=3))
        xvpool = ctx.enter_context(tc.tile_pool(name="xvpool", bufs=3))
        maskpool = ctx.enter_context(tc.tile_pool(name="maskpool", bufs=3))
        ptpool = ctx.enter_context(tc.tile_pool(name="ptpool", bufs=13))
        smalls = ctx.enter_context(tc.tile_pool(name="smalls", bufs=2))
        outpool = ctx.enter_context(tc.tile_pool(name="outpool", bufs=2))
        psp = ctx.enter_context(tc.tile_pool(name="psp", bufs=2, space="PSUM"))
        pvp = ctx.enter_context(tc.tile_pool(name="pvp", bufs=2, space="PSUM"))

        # ---- upfront DMA issues ----
        # the scalar queue's few issues all happen before its first ACTIVATE,
        # so it is free for the activation stream afterwards
        w_sb = wpool.tile([128, 3, KCP, DH4], dt.bfloat16, tag="w")
        nc.scalar.dma_start(out=w_sb[:, 1, :, :], in_=wqkv_d[:, 1, :, :])  # Wk
        wo_sb = consts.tile([128, 2, D], dt.bfloat16, tag="wo")

        xk_t = [None] * SC
        xq_t = [None] * SC
        xv_t = [None] * SC

        def dma_x(eng, lst, t, sc, halves=False):
            lst[sc] = xtv = (xtpool if t != 2 else xvpool).tile(
                [128, KCP, 512], dt.bfloat16,
                tag=("xt" if t != 2 else "xv"),
                name=f"x{'qkv'[t]}{sc}",
            )
            if halves:
                h = KCP // 2
                eng.dma_start(out=xtv[:, 0:h, :], in_=xT[t][:, sc, 0:h, :])
                eng.dma_start(out=xtv[:, h:KCP, :], in_=xT[t][:, sc, h:KCP, :])
            else:
                eng.dma_start(out=xtv[:, :, :], in_=xT[t][:, sc, :, :])

        # mask halves: (sc, h) covers kc2 in [4h, 4h+4) of both p-groups of sc
        mhalf = {}

        def dma_mask(sc, h, eng):
            mhalf[(sc, h)] = mt = maskpool.tile(
                [128, KCS // 2, 512], dt.bfloat16, tag="mk", name=f"mk{sc}_{h}"
            )
            eng.dma_start(
                out=mt[:, :, :], in_=maskT_d[:, sc, 8 * h : 8 * h + 8, :]
            )

        # Queues are FIFO and share HBM bandwidth roughly equally while all
        # are draining, so each queue carries its own stream in consumption
        # order: sync the K/Q tiles (first ones split in halves so the
        # projections start on the first 4 kc chunks), scalar only the two
        # head weights, gpsimd mask + V + late weights.
        # sync carries the PE-critical K/Q stream plus late V chunks in
        # deadline order; the short gpsimd queue (mask + first V) stops
        # competing for HBM after ~30us, freeing bandwidth for xk2/xk3
        nc.scalar.dma_start(out=w_sb[:, 0, :, :], in_=wqkv_d[:, 0, :, :])  # Wq
        dma_x(nc.sync, xk_t, 1, 0, halves=True)
        dma_x(nc.sync, xq_t, 0, 0, halves=True)
        dma_mask(0, 0, nc.gpsimd)
        nc.gpsimd.dma_start(out=w_sb[:, 2, :, :], in_=wqkv_d[:, 2, :, :])  # Wv
        dma_x(nc.gpsimd, xv_t, 2, 0)  # xv0 first: it must head the xv ring
        dma_x(nc.sync, xk_t, 1, 1)
        dma_x(nc.sync, xk_t, 1, 2)
        dma_x(nc.sync, xk_t, 1, 3)
        dma_x(nc.sync, xq_t, 0, 1)
        dma_x(nc.sync, xv_t, 2, 1)
        dma_x(nc.sync, xv_t, 2, 2)
        dma_x(nc.sync, xq_t, 0, 2)
        dma_x(nc.sync, xv_t, 2, 3)
        dma_x(nc.sync, xq_t, 0, 3)
        dma_mask(0, 1, nc.gpsimd)
        dma_mask(1, 0, nc.gpsimd)
        nc.sync.dma_start(out=wo_sb[:, :, :], in_=wo_d[:, :, :])

        # ---- persistent SBUF ----
        qt_c = [
            persist.tile([128, 2, 512], dt.bfloat16, tag=f"qt{i}", name=f"qt{i}")
            for i in range(SC)
        ]
        kt_c = [
            persist.tile([128, 2, 512], dt.bfloat16, tag=f"kt{i}", name=f"kt{i}")
            for i in range(SC)
        ]
        vaug_c = [
            persist.tile(
                [128, 4, HPC, HD + 1], dt.bfloat16, tag=f"va{i}", name=f"va{i}"
            )
            for i in range(SC)
        ]
        attnT = persist.tile([128, 2, S], dt.bfloat16, tag="attnT")
        for i in range(SC):
            nc.vector.memset(vaug_c[i][:, :, :, HD : HD + 1], 1.0)

        # PE warm-up: enough zero matmuls to ramp the clock and cover the
        # first input DMA; the real projections keep the ramp going
        warm = consts.tile([128, 640], dt.bfloat16, tag="warm")
        nc.vector.memset(warm[:, :], 0.0)
        ps_w = psp.tile([128, 1024], dt.float32, tag="ps", name="warmps")
        for i in range(6):
            nc.tensor.matmul(
                ps_w[:, 0:512],
                lhsT=warm[:, 0:128],
                rhs=warm[:, 128:640],
                start=(i == 0),
                stop=(i == 5),
            )

        def proj_qk(t, sc, cast_on_scalar=False):
            # Q^T/K^T chunk: lhsT = W chunk (stationary), rhs = X^T chunk
            src = (xq_t if t == 0 else xk_t)[sc]
            dst = (qt_c, kt_c)[t][sc]
            ps = pvp.tile([128, 1024], dt.float32, tag="pv", name=f"pj{t}{sc}")
            for kc in range(KCP):
                for m in range(2):
                    nc.tensor.matmul(
                        ps[:, m * 512 : (m + 1) * 512],
                        lhsT=w_sb[:, t, kc, m * 128 : (m + 1) * 128],
                        rhs=src[:, kc, :],
                        start=(kc == 0),
                        stop=(kc == KCP - 1),
                    )
            if cast_on_scalar:
                # pre-first-ACT head bubble: the Scalar engine is idle
                nc.scalar.activation(
                    out=dst[:, :, :],
                    in_=ps[:, :].rearrange("s (m q) -> s m q", m=2),
                    func=AF.Copy,
                )
            else:
                nc.vector.tensor_copy(
                    out=dst[:, :, :],
                    in_=ps[:, :].rearrange("s (m q) -> s m q", m=2),
                )

        # quarter-granular Q projection for the late Q2/Q3 (threads through
        # the pvp ring without monopolizing the PE for 3.5us)
        qproj_state = {}

        def proj_q_quarter(sc, quarter):
            if quarter == 0:
                qproj_state[sc] = pvp.tile(
                    [128, 1024], dt.float32, tag="pv", name=f"pjq{sc}"
                )
            ps = qproj_state[sc]
            for kc in range(2 * quarter, 2 * quarter + 2):
                for m in range(2):
                    nc.tensor.matmul(
                        ps[:, m * 512 : (m + 1) * 512],
                        lhsT=w_sb[:, 0, kc, m * 128 : (m + 1) * 128],
                        rhs=xq_t[sc][:, kc, :],
                        start=(kc == 0),
                        stop=(kc == KCP - 1),
                    )
            if quarter == 3:
                nc.vector.tensor_copy(
                    out=qt_c[sc][:, :, :],
                    in_=ps[:, :].rearrange("s (m q) -> s m q", m=2),
                )

        def proj_v_unit(sc, j, pool):
            # one V 128-row chunk: lhsT = X_v^T chunk (stationary), rhs = W_v
            po_v = pool.tile(
                [128, DH4], dt.float32,
                tag=("pv" if pool is pvp else "ps"),
                name=f"pV{sc}{j}",
            )
            for kc in range(KCP):
                nc.tensor.matmul(
                    po_v[:, :],
                    lhsT=xv_t[sc][:, kc, j * 128 : (j + 1) * 128],
                    rhs=w_sb[:, 2, kc, :],
                    start=(kc == 0),
                    stop=(kc == KCP - 1),
                )
            nc.vector.tensor_copy(
                out=vaug_c[sc][:, j, :, 0:HD],
                in_=po_v[:, :].rearrange("p (h d) -> p h d", h=4),
            )

        # slot stream: scores/exp/mask for slot s; AV trails (paced below)
        SLOTS = [
            (sc, p, kc2) for sc in range(SC) for p in range(2) for kc2 in range(8)
        ]
        pt_ring = [None] * len(SLOTS)
        pv_cur = [None, None]  # pv psum per group parity

        def scores_block(s):
            sc, p, kc2 = SLOTS[s]
            pt = ptpool.tile([128, 2, 2, 512], dt.bfloat16, tag="pt")
            pt_ring[s] = pt
            for j in range(2):
                kc = 2 * kc2 + j
                ps = psp.tile([128, 1024], dt.float32, tag="ps")
                nc.tensor.matmul(
                    ps[:, 0:512],
                    lhsT=kt_c[kc // 4][
                        0:64, p, (kc % 4) * 128 : (kc % 4 + 1) * 128
                    ],
                    rhs=qt_c[sc][0:64, p, :],
                    start=True,
                    stop=True,
                )
                nc.tensor.matmul(
                    ps[:, 512:1024],
                    lhsT=kt_c[kc // 4][
                        64:128, p, (kc % 4) * 128 : (kc % 4 + 1) * 128
                    ],
                    rhs=qt_c[sc][64:128, p, :],
                    start=True,
                    stop=True,
                    tile_position=(64, 0),
                )
                nc.scalar.activation(
                    out=pt[:, :, j, :],
                    in_=ps[:, :].rearrange("s (h q) -> s h q", h=2),
                    func=AF.Exp,
                    scale=0.125,
                )

        def mask_mul(s):
            # deferred ~2 slots behind the exp stream: AV only needs pt ~4+
            # slots later, and the lower DVE priority lets flush copies (the
            # pv ring handover) jump the queue
            sc, p, kc2 = SLOTS[s]
            pt = pt_ring[s]
            mt = mhalf[(sc, kc2 // 4)]
            msl = mt[:, 2 * (kc2 % 4) : 2 * (kc2 % 4) + 2, :]
            nc.vector.tensor_mul(
                out=pt[:, :, :, :],
                in0=pt[:, :, :, :],
                in1=msl.unsqueeze(1).broadcast_to([128, 2, 2, 512]),
            )

        def av_block(s):
            sc, p, kc2 = SLOTS[s]
            if kc2 == 0:
                pv_cur[p] = pvp.tile(
                    [HD + 1, 1024], dt.float32, tag="pv", name=f"pv{2 * sc + p}"
                )
            pv = pv_cur[p]
            pt = pt_ring[s]
            for j in range(2):
                kc = 2 * kc2 + j
                for i in range(2):
                    nc.tensor.matmul(
                        pv[:, i * 512 : (i + 1) * 512],
                        lhsT=vaug_c[kc // 4][:, kc % 4, 2 * p + i, :],
                        rhs=pt[:, i, j, :],
                        start=(kc == 0),
                        stop=(kc == KCS - 1),
                    )
            pt_ring[s] = None

        ones_sb = consts.tile([1, 64], dt.bfloat16, tag="ones")
        nc.vector.memset(ones_sb[:, :], 1.0)

        def flush(g, pe_bcast=False):
            sc, p = g // 2, g % 2
            pv = pv_cur[g % 2]
            # Stage the pv group to SBUF immediately: the two copies are
            # pv's only readers, so the PSUM slot recycles ~2us sooner than
            # if the muls (queued behind recip+broadcast) read it, and the
            # muls keep a single-PSUM-operand access pattern.
            stage = smalls.tile(
                [HD, 1024], dt.float32, tag="stage", name=f"stage{g}",
                bufs=1,
            )
            nc.vector.tensor_copy(out=stage[:, :], in_=pv[0:HD, :])
            sums_sb = smalls.tile(
                [1, 1024], dt.float32, tag="sums", name=f"sums{g}"
            )
            # custom DVE ops drop the input base-partition: stage the sums
            # row to partition 0 in SBUF before the approx reciprocal.
            nc.vector.tensor_copy(out=sums_sb[0:1, :], in_=pv[HD : HD + 1, :])
            recip_sb = smalls.tile(
                [1, 1024], dt.float32, tag="recip", name=f"recip{g}"
            )
            nc.vector.reciprocal_approx_fast(
                out=recip_sb[0:1, :], in_=sums_sb[0:1, :]
            )
            if pe_bcast:
                # tail only (PSUM is free then): rank-1 matmul broadcast is
                # ~4x faster than the GpSimd partition broadcast
                bcs = pvp.tile(
                    [64, 1024], dt.float32, tag="pv", name=f"bcsp{g}"
                )
                rb = smalls.tile(
                    [1, 1024], dt.bfloat16, tag="recipb", name=f"recipb{g}",
                    bufs=1,
                )
                nc.vector.tensor_copy(out=rb[0:1, :], in_=recip_sb[0:1, :])
                for m in range(2):
                    nc.tensor.matmul(
                        bcs[:, m * 512 : (m + 1) * 512],
                        lhsT=ones_sb[0:1, :],
                        rhs=rb[0:1, m * 512 : (m + 1) * 512],
                        start=True,
                        stop=True,
                    )
            else:
                bcs = smalls.tile(
                    [64, 1024], dt.float32, tag="bcs", name=f"bcs{g}"
                )
                nc.gpsimd.partition_broadcast(bcs[:, :], recip_sb[0:1, :])
            for i in range(2):
                nc.vector.tensor_mul(
                    out=attnT[
                        64 * i : 64 * (i + 1), p, sc * 512 : (sc + 1) * 512
                    ],
                    in0=stage[:, i * 512 : (i + 1) * 512],
                    in1=bcs[0:HD, i * 512 : (i + 1) * 512],
                )

        po_state = {}

        def po_mm(s1, pool, cs=(0, 1)):
            # out-projection matmuls for sq chunk s1, contraction parts cs
            if 0 in cs:
                po_state[s1] = pool.tile(
                    [128, 1024], dt.float32,
                    tag=("pv" if pool is pvp else "ps"),
                    name=f"po{s1}",
                )
            po = po_state[s1]
            for c in cs:
                for m in range(2):
                    nc.tensor.matmul(
                        po[:, m * 512 : (m + 1) * 512],
                        lhsT=attnT[:, c, s1 * 128 : (s1 + 1) * 128],
                        rhs=wo_sb[:, c, m * 512 : (m + 1) * 512],
                        start=(c == 0),
                        stop=(c == 1),
                    )

        def po_out(s1, dma_eng, scalar_cast=False):
            # two half casts: finer DVE granularity so flush copies (the pv
            # ring handover) never queue behind a full 1024-wide cast. At the
            # tail the idle Scalar engine takes alternate casts whole.
            po = po_state.pop(s1)
            ot = outpool.tile([128, 1024], dt.bfloat16, tag="ot")
            if scalar_cast:
                nc.scalar.activation(out=ot[:, :], in_=po[:, :], func=AF.Copy)
            else:
                nc.vector.tensor_copy(out=ot[:, 0:512], in_=po[:, 0:512])
                nc.vector.tensor_copy(out=ot[:, 512:1024], in_=po[:, 512:1024])
            dma_eng.dma_start(
                out=out_d[s1 * 128 : (s1 + 1) * 128, :], in_=ot[:, :]
            )

        def out_proj_one(s1, pool, dma_eng=None, scalar_cast=False):
            po_mm(s1, pool)
            po_out(
                s1,
                dma_eng if dma_eng is not None else nc.gpsimd,
                scalar_cast=scalar_cast,
            )

        MASK_LATE = [(1, 1), (2, 0), (2, 1), (3, 0), (3, 1)]

        def group_done(g):
            flush(g, pe_bcast=(g == 7))
            if g < len(MASK_LATE):
                dma_mask(*MASK_LATE[g], nc.sync)

        # ---- static filler schedule (see module docstring) ----
        # Each entry is (slot -> list of closures); units are sized ~<1us of
        # PE so they thread through ring idle windows without stalling ACT.
        fillers = {}

        def add_fill(s, fn):
            fillers.setdefault(s, []).append(fn)

        # V projections: 16 units across slots 8-15, alternating pvp/psp
        for scv in range(SC):
            for j in range(4):
                s_at = 8 + 2 * scv + j // 2
                pool = pvp if j % 2 == 0 else psp
                add_fill(s_at, (lambda sc_, j_, p_: lambda: proj_v_unit(sc_, j_, p_))(scv, j, pool))
        # late Q projections thread the pvp ring in quarter units
        for q in range(4):
            add_fill(16 + q, (lambda q_: lambda: proj_q_quarter(2, q_))(q))
        for q in range(4):
            add_fill(37 + q, (lambda q_: lambda: proj_q_quarter(3, q_))(q))
        # out-projections mid-stream in pvp/psp idle windows
        for s1, s_at in [(0, 27), (1, 29), (2, 31), (3, 33),
                         (4, 45), (5, 47), (6, 49),
                         (7, 53), (8, 55), (9, 57)]:
            add_fill(s_at, (lambda s1_: lambda: out_proj_one(s1_, pvp))(s1))
        for s1, s_at in [(10, 59), (11, 61)]:
            add_fill(s_at, (lambda s1_: lambda: out_proj_one(s1_, psp))(s1))

        def av_target(s):
            t = s - 10 if s < 18 else 2 * s - 28
            # wind the lag down at the end; never below 2 (mask_mul of slot
            # s-2 is the freshest one emitted before this slot's av drain)
            lag = 4 if s < 61 else max(2, 64 - s)
            return min(t, s - lag, len(SLOTS) - 1)

        # ---- program order (= scheduler priority) ----
        proj_qk(1, 0, cast_on_scalar=True)   # K0
        proj_qk(0, 0, cast_on_scalar=True)   # Q0: first scores need only kt0+qt0
        scores_block(0)
        mask_mul(0)
        scores_block(1)
        mask_mul(1)
        proj_qk(1, 1)            # K1
        scores_block(2)
        mask_mul(2)
        scores_block(3)
        mask_mul(3)
        proj_qk(1, 2)            # K2
        scores_block(4)
        mask_mul(4)
        scores_block(5)
        mask_mul(5)
        proj_qk(1, 3)            # K3
        scores_block(6)
        proj_qk(0, 1)            # Q1 (PE slack in this window)
        scores_block(7)
        av_next = 0
        for s in range(8, len(SLOTS)):
            scores_block(s)
            for fn in fillers.get(s, ()):
                fn()
            mask_mul(s - 2)
            target = av_target(s)
            while av_next <= target:
                av_block(av_next)
                sc_a, p_a, kc2_a = SLOTS[av_next]
                if kc2_a == 7:
                    group_done(2 * sc_a + p_a)
                av_next += 1
        # ---- tail ----
        # first contraction halves of the last out-proj chunks run before the
        # final flush (keeping the PE busy and at clock through it); second
        # halves + the last chunks trail it
        mask_mul(62)
        po_mm(12, psp, cs=(0,))
        mask_mul(63)
        po_mm(13, psp, cs=(0,))
        po_mm(14, pvp, cs=(0,))
        while av_next < len(SLOTS):
            av_block(av_next)
            sc_a, p_a, kc2_a = SLOTS[av_next]
            if kc2_a == 7:
                group_done(2 * sc_a + p_a)
            av_next += 1
        po_mm(12, psp, cs=(1,))
        po_out(12, nc.sync)
        po_mm(13, psp, cs=(1,))
        po_out(13, nc.scalar, scalar_cast=True)
        po_mm(14, pvp, cs=(1,))
        po_out(14, nc.sync)
        out_proj_one(15, psp, nc.scalar, scalar_cast=True)

    nc.compile()
    return nc


def _prep_inputs(query, key_, value, mask, Wq, bq, Wk, bk, Wv, bv, Wo, bo):
    bf16 = ml_dtypes.bfloat16
    f32 = np.float32

    def _xblock(x):
        # [S, D] -> X^T [D, S] -> [128p, SC, KCP, 512] (contiguous per partition)
        xt = np.asarray(x, f32).T.astype(bf16)
        return np.ascontiguousarray(
            xt.reshape(KCP, 128, SC, 512).transpose(1, 2, 0, 3)
        )

    def _mblock(mk):
        mt = np.asarray(mk).T.astype(bf16)  # maskT [sk, sq]
        return np.ascontiguousarray(
            mt.reshape(KCS, 128, SC, 512).transpose(1, 2, 0, 3)
        )

    per_batch = []
    for b in range(B):
        per_batch.append(
            {
                "xqT": _xblock(query[b]),
                "xkT": _xblock(key_[b]),
                "xvT": _xblock(value[b]),
                "maskT": _mblock(mask[b, 0]),
            }
        )
    in_maps = []
    for c in range(NCORES):
        b, hq = divmod(c, NCORES // B)
        cs = slice(DH4 * hq, DH4 * (hq + 1))
        m = dict(per_batch[b])

        def _wblock(w):
            ws = np.asarray(w, f32)[:, cs].astype(bf16)  # [D, 256]
            return ws.reshape(KCP, 128, DH4).transpose(1, 0, 2)

        m["wqkv"] = np.ascontiguousarray(
            np.stack([_wblock(Wq), _wblock(Wk), _wblock(Wv)], axis=1)
        )  # [128, 3, KCP, DH4]
        wos = np.asarray(Wo, f32)[cs, :].astype(bf16)  # [256, D]
        m["wo"] = np.ascontiguousarray(wos.reshape(2, 128, D).transpose(1, 0, 2))
        in_maps.append(m)
    return in_maps


def kernel(query, key_, value, mask, Wq, bq, Wk, bk, Wv, bv, Wo, bo):
    from concourse.bass_utils import run_bass_kernel_spmd

    if "nc" not in _CACHE:
        _CACHE["nc"] = _build_nc()
    nc = _CACHE["nc"]

    in_maps = _prep_inputs(
        query, key_, value, mask, Wq, bq, Wk, bk, Wv, bv, Wo, bo
    )
    res = run_bass_kernel_spmd(nc, in_maps, core_ids=list(range(NCORES))).results

    out = np.zeros((B, S, D), np.float32)
    for c in range(NCORES):
        out[c // (NCORES // B)] += res[c]["out"].astype(np.float32)
    out += (
        np.asarray(bv, np.float32) @ np.asarray(Wo, np.float32)
        + np.asarray(bo, np.float32)
    )[None, None, :]
    return out
